# revision 19
# baseline (speedup 1.0000x reference)
"""Trainium2 Bass kernel for nn_AdaptiveAttention (8-core SPMD).

Sharding: each core owns 2 heads (one 128-dim block of the QKV/head space)
for BOTH batches. Per core:
  - Q^T, K^T (bias folded, 1/sqrt(dk) folded into Wq) and V for its block,
  - full-T attention for its 4 (batch, head) units in the transposed-score
    layout S^T = [s, q] (lhsT=K^T slice, rhs=Q^T slice); softmax without
    max-subtraction (scores bounded ~|8|); per-batch adaptive scale applied
    via the Exp activation's scale operand; row-sums via ones-matmuls
    (column-packed); normalization folded into the AV PSUM->SBUF copy,
  - one 8-core AllToAll trades head-dim blocks for q-slices,
  - Wo matmul + residual + LayerNorm on its (batch, q-slice) of 512 rows.
Host side folds: sinusoidal PE into x, quantile MLP embeds into QKV biases,
1/sqrt(dk) into Wq/bq, bo into the residual. Core c outputs rows
[512*(c%4) : 512*(c%4+1)] of batch c//4.

All matmuls in bf16 (fp32 PSUM accumulate).
"""

import os
import numpy as np
import ml_dtypes

B, T, D = 2, 2048, 1024
H, DK = 16, 64
CPC = 128               # head-dim columns per core (2 heads)
QS = T // 4             # q-slice rows per output shard = 512
P = 128
NCORES = 8

_BF16 = ml_dtypes.bfloat16

_CACHE = {}
LAST_RESULTS = None


def _sinusoidal_pe(max_len, d_model):
    pos = np.arange(max_len)[:, None].astype(np.float32)
    div = np.exp(np.arange(0, d_model, 2).astype(np.float32) * (-np.log(10000.0) / d_model))
    pe = np.zeros((max_len, d_model), dtype=np.float32)
    pe[:, 0::2] = np.sin(pos * div)
    pe[:, 1::2] = np.cos(pos * div)
    return pe


def _build(taps=False):
    """Build + compile the SPMD Bass graph (one NEFF, runs on all 8 cores)."""
    import concourse.bass as bass
    import concourse.mybir as mybir
    import concourse.tile as tile
    from concourse import bacc

    f32 = mybir.dt.float32
    bf = mybir.dt.bfloat16
    Exp = mybir.ActivationFunctionType.Exp
    Sqrt = mybir.ActivationFunctionType.Sqrt
    sub = mybir.AluOpType.subtract
    mult = mybir.AluOpType.mult

    nc = bacc.Bacc("TRN2", target_bir_lowering=False, debug=False, num_devices=NCORES)

    xt_d = nc.dram_tensor("xt", [B, D, T], bf, kind="ExternalInput")
    xres_d = nc.dram_tensor("xres", [QS, D], f32, kind="ExternalInput")
    wq_d = nc.dram_tensor("wq", [D, CPC], bf, kind="ExternalInput")
    wk_d = nc.dram_tensor("wk", [D, CPC], bf, kind="ExternalInput")
    wv_d = nc.dram_tensor("wv", [D, CPC], bf, kind="ExternalInput")
    bq_d = nc.dram_tensor("bq", [B, CPC], f32, kind="ExternalInput")
    bk_d = nc.dram_tensor("bk", [B, CPC], f32, kind="ExternalInput")
    bv_d = nc.dram_tensor("bv", [B, CPC], f32, kind="ExternalInput")
    sc_d = nc.dram_tensor("scales", [B], f32, kind="ExternalInput")
    wo_d = nc.dram_tensor("wo", [D, D], bf, kind="ExternalInput")
    lng_d = nc.dram_tensor("lng", [D], f32, kind="ExternalInput")
    lnb_d = nc.dram_tensor("lnb", [D], f32, kind="ExternalInput")
    out_d = nc.dram_tensor("out", [QS, D], f32, kind="ExternalOutput")
    a2a_i = nc.dram_tensor("a2a_i", [NCORES, CPC, QS], bf, kind="Internal")
    a2a_o = nc.dram_tensor("a2a_o", [NCORES, CPC, QS], bf, kind="Internal")
    tap_d = {}
    if taps:
        tap_d["qt"] = nc.dram_tensor("tap_qt", [P, B, T], bf, kind="ExternalOutput")
        tap_d["kt"] = nc.dram_tensor("tap_kt", [P, B, T], bf, kind="ExternalOutput")
        tap_d["v"] = nc.dram_tensor("tap_v", [P, B, 16, 2, 65], bf, kind="ExternalOutput")
        tap_d["att"] = nc.dram_tensor("tap_att", [64, B, 2, T], bf, kind="ExternalOutput")
        tap_d["attf"] = nc.dram_tensor("tap_attf", [P, 8, QS], bf, kind="ExternalOutput")
        tap_d["rs"] = nc.dram_tensor("tap_rs", [4, 4, 512], f32, kind="ExternalOutput")

    def bcast_ap(src, nparts):
        """Partition-broadcast DMA source AP from a 1-partition AP."""
        return bass.AP(
            tensor=src.tensor,
            offset=src.offset,
            ap=[[0, nparts]] + [list(d) for d in src.ap[1:]],
        )

    with tile.TileContext(nc) as tc:
        with tc.tile_pool(name="const", bufs=1) as const:
            # ---- load inputs ----
            xt_sb = const.tile([P, B, 8, T], bf)
            xt_ap = xt_d.ap()
            for b in range(B):
                for k in range(8):
                    nc.sync.dma_start(out=xt_sb[:, b, k, :], in_=xt_ap[b, k * P:(k + 1) * P, :])

            w_sbs = {}
            for name, wd in (("q", wq_d), ("k", wk_d), ("v", wv_d)):
                w_sb = const.tile([P, 8, CPC], bf, name=f"w{name}_sb")
                wap = wd.ap()
                for k in range(8):
                    nc.sync.dma_start(out=w_sb[:, k, :], in_=wap[k * P:(k + 1) * P, :])
                w_sbs[name] = w_sb

            wo_sb = const.tile([P, 8, D], bf)
            wo_ap = wo_d.ap()
            for k in range(8):
                nc.sync.dma_start(out=wo_sb[:, k, :], in_=wo_ap[k * P:(k + 1) * P, :])

            bq_sb = const.tile([P, B], f32)
            bk_sb = const.tile([P, B], f32)
            nc.sync.dma_start(out=bq_sb[:], in_=bq_d.ap().rearrange("b p -> p b"))
            nc.sync.dma_start(out=bk_sb[:], in_=bk_d.ap().rearrange("b p -> p b"))
            bv_bc = const.tile([P, B, CPC], f32)
            for b in range(B):
                nc.sync.dma_start(out=bv_bc[:, b, :], in_=bcast_ap(bv_d.ap()[b:b + 1, :], P))
            scale_sb = const.tile([P, B], f32)
            nc.sync.dma_start(out=scale_sb[:], in_=bcast_ap(sc_d.ap()[None, :], P))
            lng_b = const.tile([P, D], f32)
            lnb_b = const.tile([P, D], f32)
            nc.sync.dma_start(out=lng_b[:], in_=bcast_ap(lng_d.ap()[None, :], P))
            nc.sync.dma_start(out=lnb_b[:], in_=bcast_ap(lnb_d.ap()[None, :], P))
            xres_sb = const.tile([P, 4, D], f32)
            xres_ap = xres_d.ap()
            for m in range(4):
                nc.sync.dma_start(out=xres_sb[:, m, :], in_=xres_ap[m * P:(m + 1) * P, :])

            eps_sb = const.tile([P, 1], f32)
            nc.vector.memset(eps_sb[:], 1e-5)

            QT_sb = const.tile([P, B, T], bf)
            KT_sb = const.tile([P, B, T], bf)
            # V augmented with a ones-column per head: free layout [b, schunk, hh, 65]
            # col 64 == 1.0 so the AV matmul's output row 64 is the exp row-sum.
            V_sb = const.tile([P, B, 16, 2, 65], bf)
            nc.vector.memset(V_sb[:], 1.0)
            attT_sb = const.tile([64, B, 2, T], bf)
            attTf_sb = const.tile([P, 8, QS], bf)

            # ---- phase 1: projections ----
            with tc.tile_pool(name="qk_ps", bufs=4, space="PSUM") as qk_ps, \
                 tc.tile_pool(name="v_ps", bufs=4, space="PSUM") as v_ps:
                for w_sb, b_sb, dst in ((w_sbs["q"], bq_sb, QT_sb), (w_sbs["k"], bk_sb, KT_sb)):
                    for b in range(B):
                        for n in range(4):
                            ps = qk_ps.tile([P, 512], f32, tag="qkps")
                            for k in range(8):
                                nc.tensor.matmul(
                                    ps[:],
                                    lhsT=w_sb[:, k, :],
                                    rhs=xt_sb[:, b, k, n * 512:(n + 1) * 512],
                                    start=(k == 0), stop=(k == 7),
                                )
                            nc.vector.tensor_scalar_add(
                                out=dst[:, b, n * 512:(n + 1) * 512],
                                in0=ps[:], scalar1=b_sb[:, b:b + 1],
                            )
                for b in range(B):
                    for mt in range(16):
                        ps = v_ps.tile([P, CPC], f32, tag="vps")
                        for k in range(8):
                            nc.tensor.matmul(
                                ps[:],
                                lhsT=xt_sb[:, b, k, mt * P:(mt + 1) * P],
                                rhs=w_sbs["v"][:, k, :],
                                start=(k == 0), stop=(k == 7),
                            )
                        for hh in range(2):
                            hsl = slice(64 * hh, 64 * (hh + 1))
                            nc.vector.tensor_add(
                                out=V_sb[:, b, mt, hh, 0:64],
                                in0=ps[:, hsl], in1=bv_bc[:, b, hsl],
                            )

            # ---- phase 2: attention (S^T layout) ----
            with tc.tile_pool(name="sp_ps", bufs=4, space="PSUM") as sp_ps, \
                 tc.tile_pool(name="av_ps", bufs=4, space="PSUM") as av_ps, \
                 tc.tile_pool(name="et_pool", bufs=4) as et_pool, \
                 tc.tile_pool(name="rc_pool", bufs=2) as rc_pool, \
                 tc.tile_pool(name="rcd_pool", bufs=2, space="DRAM") as rcd_pool, \
                 tc.tile_pool(name="rb_pool", bufs=8) as rb_pool:
                for qb in range(4):
                    qsl = slice(qb * 512, (qb + 1) * 512)
                    avs = [av_ps.tile([65, 512], f32, tag="av", name=f"av{qb}_{u}")
                           for u in range(4)]
                    for sc in range(16):
                        for b in range(B):
                            for hh in range(2):
                                u = 2 * b + hh
                                hsl = slice(64 * hh, 64 * (hh + 1))
                                sp = sp_ps.tile([P, 512], f32, tag="sp")
                                nc.tensor.matmul(
                                    sp[:],
                                    lhsT=KT_sb[hsl, b, sc * P:(sc + 1) * P],
                                    rhs=QT_sb[hsl, b, qsl],
                                    start=True, stop=True,
                                )
                                et = et_pool.tile([P, 512], bf, tag="et")
                                nc.scalar.activation(
                                    out=et[:], in_=sp[:], func=Exp,
                                    scale=scale_sb[:, b:b + 1],
                                )
                                nc.tensor.matmul(
                                    avs[u][:],
                                    lhsT=V_sb[:, b, sc, hh, :],
                                    rhs=et[:],
                                    start=(sc == 0), stop=(sc == 15),
                                )
                    rc = rc_pool.tile([P, 4, 512], f32, tag="rc")
                    for u in range(4):
                        nc.vector.reciprocal(
                            out=rc[64:65, u, :], in_=avs[u][64:65, :]
                        )
                    rcd = rcd_pool.tile([4, 512], f32, tag="rcd")
                    nc.sync.dma_start(out=rcd[:], in_=rc[64:65, :, :])
                    if taps:
                        nc.sync.dma_start(out=tap_d["rs"].ap()[qb], in_=rcd[:])
                    for b in range(B):
                        for hh in range(2):
                            u = 2 * b + hh
                            rb = rb_pool.tile([64, 512], f32, tag="rb")
                            nc.sync.dma_start(
                                out=rb[:], in_=bcast_ap(rcd[u:u + 1, :], 64),
                            )
                            nc.vector.tensor_mul(
                                out=attT_sb[:, b, hh, qsl], in0=avs[u][0:64, :], in1=rb[:],
                            )
                    for j2 in range(2):  # the two a2a slots covering this qb
                        j = j2 * 4 + qb
                        for hh in range(2):
                            nc.sync.dma_start(
                                out=a2a_i.ap()[j, 64 * hh:64 * (hh + 1), :],
                                in_=attT_sb[:, j2, hh, qsl],
                            )

            if taps:
                for b in range(B):
                    nc.sync.dma_start(out=tap_d["qt"].ap()[:, b, :], in_=QT_sb[:, b, :])
                    nc.sync.dma_start(out=tap_d["kt"].ap()[:, b, :], in_=KT_sb[:, b, :])
                    for hh in range(2):
                        nc.sync.dma_start(
                            out=tap_d["att"].ap()[:, b, hh, :], in_=attT_sb[:, b, hh, :]
                        )
                    for mt in range(16):
                        nc.sync.dma_start(
                            out=tap_d["v"].ap()[:, b, mt, :, :], in_=V_sb[:, b, mt, :, :]
                        )

            # ---- phase 3: all-to-all + Wo + residual + LN ----
            nc.gpsimd.collective_compute(
                "AllToAll",
                mybir.AluOpType.bypass,
                replica_groups=[list(range(NCORES))],
                ins=[a2a_i.ap()],
                outs=[a2a_o.ap()],
            )
            for j in range(8):
                nc.sync.dma_start(out=attTf_sb[:, j, :], in_=a2a_o.ap()[j, :, :])
            if taps:
                for j in range(8):
                    nc.sync.dma_start(out=tap_d["attf"].ap()[:, j, :], in_=attTf_sb[:, j, :])

            with tc.tile_pool(name="wo_ps", bufs=4, space="PSUM") as wo_ps, \
                 tc.tile_pool(name="y_pool", bufs=3) as y_pool, \
                 tc.tile_pool(name="stat", bufs=4) as stat:
                for m in range(4):
                    y = y_pool.tile([P, D], f32, tag="y")
                    for n in range(2):
                        nsl = slice(n * 512, (n + 1) * 512)
                        ps = wo_ps.tile([P, 512], f32, tag="wops")
                        for k in range(8):
                            nc.tensor.matmul(
                                ps[:],
                                lhsT=attTf_sb[:, k, m * P:(m + 1) * P],
                                rhs=wo_sb[:, k, nsl],
                                start=(k == 0), stop=(k == 7),
                            )
                        nc.vector.tensor_add(out=y[:, nsl], in0=ps[:], in1=xres_sb[:, m, nsl])
                    st = stat.tile([P, 2, 6], f32, tag="st")
                    nc.vector.bn_stats(out=st[:, 0, :], in_=y[:, 0:512])
                    nc.vector.bn_stats(out=st[:, 1, :], in_=y[:, 512:1024])
                    mv = stat.tile([P, 2], f32, tag="mv")
                    nc.vector.bn_aggr(out=mv[:], in_=st[:])
                    std = stat.tile([P, 1], f32, tag="std")
                    nc.scalar.activation(out=std[:], in_=mv[:, 1:2], func=Sqrt, bias=eps_sb[:])
                    rstd = stat.tile([P, 1], f32, tag="rstd")
                    nc.vector.reciprocal(out=rstd[:], in_=std[:])
                    nc.vector.tensor_scalar(
                        out=y[:], in0=y[:], scalar1=mv[:, 0:1], scalar2=rstd[:],
                        op0=sub, op1=mult,
                    )
                    nc.vector.tensor_mul(out=y[:], in0=y[:], in1=lng_b[:])
                    nc.vector.tensor_add(out=y[:], in0=y[:], in1=lnb_b[:])
                    nc.sync.dma_start(out=out_d.ap()[m * P:(m + 1) * P, :], in_=y[:])

    nc.compile()
    return nc


def _prep_inputs(x, quantile, quantile_importance,
                 Wq, bq, Wk, bk, Wv, bv, Wo, bo,
                 qpq_w1, qpq_b1, qpq_w2, qpq_b2,
                 qpk_w1, qpk_b1, qpk_w2, qpk_b2,
                 qpv_w1, qpv_b1, qpv_w2, qpv_b2,
                 ln_g, ln_b):
    pe = _sinusoidal_pe(T, D)
    xp = x.astype(np.float32) + pe[None]

    q = quantile.astype(np.float32)

    def mlp(w1, b1, w2, b2):
        return np.maximum(q @ w1 + b1, 0.0) @ w2 + b2

    q_embed = mlp(qpq_w1, qpq_b1, qpq_w2, qpq_b2)
    k_embed = mlp(qpk_w1, qpk_b1, qpk_w2, qpk_b2)
    v_embed = mlp(qpv_w1, qpv_b1, qpv_w2, qpv_b2)

    buf = quantile_importance.astype(np.float32)
    idx = np.clip((q[:, 0] * 100).astype(np.int32), 0, 99)
    mx = buf.max()
    imp = buf[idx]
    imp = np.where(mx > 0, imp / mx, imp)
    scales = (1.0 + imp).astype(np.float32)          # [B], applied inside Exp
    rdk = np.float32(1.0 / np.sqrt(DK))              # folded into Wq/bq

    xt_all = np.ascontiguousarray(np.transpose(xp, (0, 2, 1))).astype(_BF16)  # [B, D, T]
    in_maps = []
    for c in range(NCORES):
        b, r = divmod(c, 4)
        cols = slice(c * CPC, (c + 1) * CPC)
        qsl = slice(r * QS, (r + 1) * QS)
        in_maps.append({
            "xt": xt_all,
            "xres": np.ascontiguousarray(xp[b, qsl] + bo[None, :]).astype(np.float32),
            "wq": np.ascontiguousarray(Wq[:, cols] * rdk).astype(_BF16),
            "wk": np.ascontiguousarray(Wk[:, cols]).astype(_BF16),
            "wv": np.ascontiguousarray(Wv[:, cols]).astype(_BF16),
            "bq": np.ascontiguousarray((bq[None, cols] + q_embed[:, cols]) * rdk).astype(np.float32),
            "bk": np.ascontiguousarray(bk[None, cols] + k_embed[:, cols]).astype(np.float32),
            "bv": np.ascontiguousarray(bv[None, cols] + v_embed[:, cols]).astype(np.float32),
            "scales": scales,
            "wo": Wo.astype(_BF16),
            "lng": ln_g.astype(np.float32),
            "lnb": ln_b.astype(np.float32),
        })
    return in_maps


def kernel(**inputs):
    global LAST_RESULTS
    from concourse import bass_utils

    inputs = {k: np.asarray(v) for k, v in inputs.items()}
    if "nc" not in _CACHE:
        _CACHE["nc"] = _build()
    nc = _CACHE["nc"]

    in_maps = _prep_inputs(**inputs)
    res = bass_utils.run_bass_kernel_spmd(nc, in_maps, core_ids=list(range(NCORES)))
    LAST_RESULTS = res

    out = np.zeros((B, T, D), np.float32)
    for c in range(NCORES):
        b, r = divmod(c, 4)
        out[b, r * QS:(r + 1) * QS, :] = res.results[c]["out"]
    return out


# revision 26
# speedup vs baseline: 1.1798x; 1.1798x over previous
"""Trainium2 Bass kernel for nn_AdaptiveAttention (8-core SPMD).

Sharding: each core owns 2 heads (one 128-dim block of the QKV/head space)
for BOTH batches. Per core:
  - Q^T, K^T (bias folded, 1/sqrt(dk) folded into Wq) and V for its block,
  - full-T attention for its 4 (batch, head) units in the transposed-score
    layout S^T = [s, q] (lhsT=K^T slice, rhs=Q^T slice); softmax without
    max-subtraction (scores bounded ~|8|); per-batch adaptive scale applied
    via the Exp activation's scale operand; row-sums via ones-matmuls
    (column-packed); normalization folded into the AV PSUM->SBUF copy,
  - one 8-core AllToAll trades head-dim blocks for q-slices,
  - Wo matmul + residual + LayerNorm on its (batch, q-slice) of 512 rows.
Host side folds: sinusoidal PE into x, quantile MLP embeds into QKV biases,
1/sqrt(dk) into Wq/bq, bo into the residual. Core c outputs rows
[512*(c%4) : 512*(c%4+1)] of batch c//4.

All matmuls in bf16 (fp32 PSUM accumulate).
"""

import os
import numpy as np
import ml_dtypes

B, T, D = 2, 2048, 1024
H, DK = 16, 64
CPC = 128               # head-dim columns per core (2 heads)
QS = T // 4             # q-slice rows per output shard = 512
P = 128
NCORES = 8

_BF16 = ml_dtypes.bfloat16

_CACHE = {}
LAST_RESULTS = None


def _sinusoidal_pe(max_len, d_model):
    pos = np.arange(max_len)[:, None].astype(np.float32)
    div = np.exp(np.arange(0, d_model, 2).astype(np.float32) * (-np.log(10000.0) / d_model))
    pe = np.zeros((max_len, d_model), dtype=np.float32)
    pe[:, 0::2] = np.sin(pos * div)
    pe[:, 1::2] = np.cos(pos * div)
    return pe


def _build(taps=False):
    """Build + compile the SPMD Bass graph (one NEFF, runs on all 8 cores)."""
    import concourse.bass as bass
    import concourse.mybir as mybir
    import concourse.tile as tile
    from concourse import bacc

    f32 = mybir.dt.float32
    bf = mybir.dt.bfloat16
    Exp = mybir.ActivationFunctionType.Exp
    Sqrt = mybir.ActivationFunctionType.Sqrt
    sub = mybir.AluOpType.subtract
    mult = mybir.AluOpType.mult

    nc = bacc.Bacc("TRN2", target_bir_lowering=False, debug=False, num_devices=NCORES)

    xt_d = nc.dram_tensor("xt", [B, D, T], bf, kind="ExternalInput")
    xres_d = nc.dram_tensor("xres", [QS, D], f32, kind="ExternalInput")
    wq_d = nc.dram_tensor("wq", [D, CPC], bf, kind="ExternalInput")
    wk_d = nc.dram_tensor("wk", [D, CPC], bf, kind="ExternalInput")
    wv_d = nc.dram_tensor("wv", [D, CPC], bf, kind="ExternalInput")
    bq_d = nc.dram_tensor("bq", [B, CPC], f32, kind="ExternalInput")
    bk_d = nc.dram_tensor("bk", [B, CPC], f32, kind="ExternalInput")
    bv_d = nc.dram_tensor("bv", [B, CPC], f32, kind="ExternalInput")
    sc_d = nc.dram_tensor("scales", [B], f32, kind="ExternalInput")
    wo_d = nc.dram_tensor("wo", [D, D], bf, kind="ExternalInput")
    lng_d = nc.dram_tensor("lng", [D], f32, kind="ExternalInput")
    lnb_d = nc.dram_tensor("lnb", [D], f32, kind="ExternalInput")
    out_d = nc.dram_tensor("out", [QS, D], f32, kind="ExternalOutput")
    a2a_i = nc.dram_tensor("a2a_i", [NCORES, CPC, QS], bf, kind="Internal")
    a2a_o = nc.dram_tensor("a2a_o", [NCORES, CPC, QS], bf, kind="Internal")
    tap_d = {}
    if taps:
        tap_d["qt"] = nc.dram_tensor("tap_qt", [P, B, T], bf, kind="ExternalOutput")
        tap_d["kt"] = nc.dram_tensor("tap_kt", [P, B, T], bf, kind="ExternalOutput")
        tap_d["v"] = nc.dram_tensor("tap_v", [P, B, 16, 2, 65], bf, kind="ExternalOutput")
        tap_d["att"] = nc.dram_tensor("tap_att", [64, B, 2, T], bf, kind="ExternalOutput")
        tap_d["attf"] = nc.dram_tensor("tap_attf", [P, 8, QS], bf, kind="ExternalOutput")
        tap_d["rs"] = nc.dram_tensor("tap_rs", [4, 4, 512], f32, kind="ExternalOutput")

    def bcast_ap(src, nparts):
        """Partition-broadcast DMA source AP from a 1-partition AP."""
        return bass.AP(
            tensor=src.tensor,
            offset=src.offset,
            ap=[[0, nparts]] + [list(d) for d in src.ap[1:]],
        )

    with tile.TileContext(nc) as tc:
        with tc.tile_pool(name="const", bufs=1) as const:
            # ---- load inputs (spread across engine DMA queues) ----
            qeng = [nc.sync, nc.scalar, nc.gpsimd, nc.sync]
            xt_sb = const.tile([P, B, 8, T], bf)
            xt_ap = xt_d.ap()
            for b in range(B):
                for k in range(8):
                    qeng[(b * 8 + k) % 4].dma_start(
                        out=xt_sb[:, b, k, :], in_=xt_ap[b, k * P:(k + 1) * P, :]
                    )

            w_sbs = {}
            for wi, (name, wd) in enumerate((("q", wq_d), ("k", wk_d), ("v", wv_d))):
                w_sb = const.tile([P, 8, CPC], bf, name=f"w{name}_sb")
                wap = wd.ap()
                for k in range(8):
                    qeng[(wi * 8 + k) % 4].dma_start(
                        out=w_sb[:, k, :], in_=wap[k * P:(k + 1) * P, :]
                    )
                w_sbs[name] = w_sb

            wo_sb = const.tile([P, 8, D], bf)
            wo_ap = wo_d.ap()
            for k in range(8):
                qeng[k % 4].dma_start(out=wo_sb[:, k, :], in_=wo_ap[k * P:(k + 1) * P, :])

            bq_sb = const.tile([P, B], f32)
            bk_sb = const.tile([P, B], f32)
            nc.sync.dma_start(out=bq_sb[:], in_=bq_d.ap().rearrange("b p -> p b"))
            nc.sync.dma_start(out=bk_sb[:], in_=bk_d.ap().rearrange("b p -> p b"))
            bv_bc = const.tile([P, B, CPC], f32)
            for b in range(B):
                nc.sync.dma_start(out=bv_bc[:, b, :], in_=bcast_ap(bv_d.ap()[b:b + 1, :], P))
            scale_sb = const.tile([P, B], f32)
            nc.sync.dma_start(out=scale_sb[:], in_=bcast_ap(sc_d.ap()[None, :], P))
            lng_b = const.tile([P, D], f32)
            lnb_b = const.tile([P, D], f32)
            nc.sync.dma_start(out=lng_b[:], in_=bcast_ap(lng_d.ap()[None, :], P))
            nc.sync.dma_start(out=lnb_b[:], in_=bcast_ap(lnb_d.ap()[None, :], P))
            xres_sb = const.tile([P, 4, D], f32)
            xres_ap = xres_d.ap()
            for m in range(4):
                nc.sync.dma_start(out=xres_sb[:, m, :], in_=xres_ap[m * P:(m + 1) * P, :])

            eps_sb = const.tile([P, 1], f32)
            nc.vector.memset(eps_sb[:], 1e-5)

            QT_sb = const.tile([P, B, T], bf)
            KT_sb = const.tile([P, B, T], bf)
            # V augmented with a ones-column per head: free layout [b, schunk, hh, 65]
            # col 64 == 1.0 so the AV matmul's output row 64 is the exp row-sum.
            V_sb = const.tile([P, B, 16, 2, 65], bf)
            nc.vector.memset(V_sb[:], 1.0)
            attT_sb = const.tile([64, B, 2, T], bf)
            attTf_sb = const.tile([P, 8, QS], bf)

            # ---- phase 1: projections ----
            with tc.tile_pool(name="qk_ps", bufs=4, space="PSUM") as qk_ps, \
                 tc.tile_pool(name="v_ps", bufs=4, space="PSUM") as v_ps:
                for w_sb, b_sb, dst in ((w_sbs["q"], bq_sb, QT_sb), (w_sbs["k"], bk_sb, KT_sb)):
                    for b in range(B):
                        for n in range(4):
                            ps = qk_ps.tile([P, 512], f32, tag="qkps")
                            for k in range(8):
                                nc.tensor.matmul(
                                    ps[:],
                                    lhsT=w_sb[:, k, :],
                                    rhs=xt_sb[:, b, k, n * 512:(n + 1) * 512],
                                    start=(k == 0), stop=(k == 7),
                                )
                            nc.vector.tensor_scalar_add(
                                out=dst[:, b, n * 512:(n + 1) * 512],
                                in0=ps[:], scalar1=b_sb[:, b:b + 1],
                            )
                for b in range(B):
                    for mt in range(16):
                        ps = v_ps.tile([P, CPC], f32, tag="vps")
                        for k in range(8):
                            nc.tensor.matmul(
                                ps[:],
                                lhsT=xt_sb[:, b, k, mt * P:(mt + 1) * P],
                                rhs=w_sbs["v"][:, k, :],
                                start=(k == 0), stop=(k == 7),
                            )
                        for hh in range(2):
                            hsl = slice(64 * hh, 64 * (hh + 1))
                            nc.vector.tensor_add(
                                out=V_sb[:, b, mt, hh, 0:64],
                                in0=ps[:, hsl], in1=bv_bc[:, b, hsl],
                            )

            # ---- phase 2: attention (S^T layout) ----
            with tc.tile_pool(name="sp_ps", bufs=2, space="PSUM") as sp_ps, \
                 tc.tile_pool(name="av_ps", bufs=4, space="PSUM") as av_ps, \
                 tc.tile_pool(name="et_pool", bufs=4) as et_pool, \
                 tc.tile_pool(name="avs_pool", bufs=6) as avs_pool, \
                 tc.tile_pool(name="rc_pool", bufs=2) as rc_pool, \
                 tc.tile_pool(name="rcd_pool", bufs=2, space="DRAM") as rcd_pool, \
                 tc.tile_pool(name="rb_pool", bufs=8) as rb_pool:
                for qb in range(4):
                    qsl = slice(qb * 512, (qb + 1) * 512)
                    avs = [av_ps.tile([65, 512], f32, tag="av", name=f"av{qb}_{u}")
                           for u in range(4)]
                    for sc in range(16):
                        for b in range(B):
                            # one 2-bank PSUM tile holds both heads' scores;
                            # the two matmuls use distinct row groups (h0 rows
                            # 0-63, h1 rows 64-127) and run concurrently.
                            sp = sp_ps.tile([P, 2, 512], f32, tag="sp")
                            nc.tensor.matmul(
                                sp[:, 0, :],
                                lhsT=KT_sb[0:64, b, sc * P:(sc + 1) * P],
                                rhs=QT_sb[0:64, b, qsl],
                                start=True, stop=True,
                            )
                            nc.tensor.matmul(
                                sp[:, 1, :],
                                lhsT=KT_sb[64:128, b, sc * P:(sc + 1) * P],
                                rhs=QT_sb[64:128, b, qsl],
                                start=True, stop=True,
                            )
                            et = et_pool.tile([P, 2, 512], bf, tag="et")
                            nc.scalar.activation(
                                out=et[:], in_=sp[:], func=Exp,
                                scale=scale_sb[:, b:b + 1],
                            )
                            for hh in range(2):
                                nc.tensor.matmul(
                                    avs[2 * b + hh][:],
                                    lhsT=V_sb[:, b, sc, hh, :],
                                    rhs=et[:, hh, :],
                                    start=(sc == 0), stop=(sc == 15),
                                )
                    # decouple: drain AV PSUM to SBUF so the next q-block's
                    # matmuls can reuse the banks; normalization happens from
                    # SBUF off the PE critical path.
                    av_sb = [avs_pool.tile([65, 512], f32, tag="avsb", name=f"avsb{qb}_{u}")
                             for u in range(4)]
                    for u in range(4):
                        nc.vector.tensor_copy(out=av_sb[u][:], in_=avs[u][:])
                    rc = rc_pool.tile([P, 4, 512], bf, tag="rc")
                    with nc.allow_low_precision(reason="softmax recip in bf16 is fine at 2e-2 tol"):
                        for u in range(4):
                            nc.vector.reciprocal(
                                out=rc[64:65, u, :], in_=av_sb[u][64:65, :]
                            )
                    rcd = rcd_pool.tile([4, 512], bf, tag="rcd")
                    nc.sync.dma_start(out=rcd[:], in_=rc[64:65, :, :])
                    if taps:
                        nc.sync.dma_start(out=tap_d["rs"].ap()[qb], in_=rcd[:])
                    for b in range(B):
                        for hh in range(2):
                            u = 2 * b + hh
                            rb = rb_pool.tile([64, 512], bf, tag="rb")
                            nc.sync.dma_start(
                                out=rb[:], in_=bcast_ap(rcd[u:u + 1, :], 64),
                            )
                            nc.vector.tensor_mul(
                                out=attT_sb[:, b, hh, qsl], in0=av_sb[u][0:64, :], in1=rb[:],
                            )
                    for j2 in range(2):  # the two a2a slots covering this qb
                        j = j2 * 4 + qb
                        for hh in range(2):
                            nc.sync.dma_start(
                                out=a2a_i.ap()[j, 64 * hh:64 * (hh + 1), :],
                                in_=attT_sb[:, j2, hh, qsl],
                            )

            if taps:
                for b in range(B):
                    nc.sync.dma_start(out=tap_d["qt"].ap()[:, b, :], in_=QT_sb[:, b, :])
                    nc.sync.dma_start(out=tap_d["kt"].ap()[:, b, :], in_=KT_sb[:, b, :])
                    for hh in range(2):
                        nc.sync.dma_start(
                            out=tap_d["att"].ap()[:, b, hh, :], in_=attT_sb[:, b, hh, :]
                        )
                    for mt in range(16):
                        nc.sync.dma_start(
                            out=tap_d["v"].ap()[:, b, mt, :, :], in_=V_sb[:, b, mt, :, :]
                        )

            # ---- phase 3: all-to-all + Wo + residual + LN ----
            nc.gpsimd.collective_compute(
                "AllToAll",
                mybir.AluOpType.bypass,
                replica_groups=[list(range(NCORES))],
                ins=[a2a_i.ap()],
                outs=[a2a_o.ap()],
            )
            for j in range(8):
                qeng[j % 4].dma_start(out=attTf_sb[:, j, :], in_=a2a_o.ap()[j, :, :])
            if taps:
                for j in range(8):
                    nc.sync.dma_start(out=tap_d["attf"].ap()[:, j, :], in_=attTf_sb[:, j, :])

            with tc.tile_pool(name="wo_ps", bufs=4, space="PSUM") as wo_ps, \
                 tc.tile_pool(name="y_pool", bufs=3) as y_pool, \
                 tc.tile_pool(name="stat", bufs=4) as stat:
                for m in range(4):
                    y = y_pool.tile([P, D], f32, tag="y")
                    for n in range(2):
                        nsl = slice(n * 512, (n + 1) * 512)
                        ps = wo_ps.tile([P, 512], f32, tag="wops")
                        for k in range(8):
                            nc.tensor.matmul(
                                ps[:],
                                lhsT=attTf_sb[:, k, m * P:(m + 1) * P],
                                rhs=wo_sb[:, k, nsl],
                                start=(k == 0), stop=(k == 7),
                            )
                        nc.vector.tensor_add(out=y[:, nsl], in0=ps[:], in1=xres_sb[:, m, nsl])
                    st = stat.tile([P, 2, 6], f32, tag="st")
                    nc.vector.bn_stats(out=st[:, 0, :], in_=y[:, 0:512])
                    nc.vector.bn_stats(out=st[:, 1, :], in_=y[:, 512:1024])
                    mv = stat.tile([P, 2], f32, tag="mv")
                    nc.vector.bn_aggr(out=mv[:], in_=st[:])
                    std = stat.tile([P, 1], f32, tag="std")
                    nc.scalar.activation(out=std[:], in_=mv[:, 1:2], func=Sqrt, bias=eps_sb[:])
                    rstd = stat.tile([P, 1], f32, tag="rstd")
                    nc.vector.reciprocal(out=rstd[:], in_=std[:])
                    nc.vector.tensor_scalar(
                        out=y[:], in0=y[:], scalar1=mv[:, 0:1], scalar2=rstd[:],
                        op0=sub, op1=mult,
                    )
                    nc.vector.tensor_mul(out=y[:], in0=y[:], in1=lng_b[:])
                    nc.vector.tensor_add(out=y[:], in0=y[:], in1=lnb_b[:])
                    qeng[m % 4].dma_start(out=out_d.ap()[m * P:(m + 1) * P, :], in_=y[:])

    nc.compile()
    return nc


def _prep_inputs(x, quantile, quantile_importance,
                 Wq, bq, Wk, bk, Wv, bv, Wo, bo,
                 qpq_w1, qpq_b1, qpq_w2, qpq_b2,
                 qpk_w1, qpk_b1, qpk_w2, qpk_b2,
                 qpv_w1, qpv_b1, qpv_w2, qpv_b2,
                 ln_g, ln_b):
    pe = _sinusoidal_pe(T, D)
    xp = x.astype(np.float32) + pe[None]

    q = quantile.astype(np.float32)

    def mlp(w1, b1, w2, b2):
        return np.maximum(q @ w1 + b1, 0.0) @ w2 + b2

    q_embed = mlp(qpq_w1, qpq_b1, qpq_w2, qpq_b2)
    k_embed = mlp(qpk_w1, qpk_b1, qpk_w2, qpk_b2)
    v_embed = mlp(qpv_w1, qpv_b1, qpv_w2, qpv_b2)

    buf = quantile_importance.astype(np.float32)
    idx = np.clip((q[:, 0] * 100).astype(np.int32), 0, 99)
    mx = buf.max()
    imp = buf[idx]
    imp = np.where(mx > 0, imp / mx, imp)
    scales = (1.0 + imp).astype(np.float32)          # [B], applied inside Exp
    rdk = np.float32(1.0 / np.sqrt(DK))              # folded into Wq/bq

    xt_all = np.ascontiguousarray(np.transpose(xp, (0, 2, 1))).astype(_BF16)  # [B, D, T]
    in_maps = []
    for c in range(NCORES):
        b, r = divmod(c, 4)
        cols = slice(c * CPC, (c + 1) * CPC)
        qsl = slice(r * QS, (r + 1) * QS)
        in_maps.append({
            "xt": xt_all,
            "xres": np.ascontiguousarray(xp[b, qsl] + bo[None, :]).astype(np.float32),
            "wq": np.ascontiguousarray(Wq[:, cols] * rdk).astype(_BF16),
            "wk": np.ascontiguousarray(Wk[:, cols]).astype(_BF16),
            "wv": np.ascontiguousarray(Wv[:, cols]).astype(_BF16),
            "bq": np.ascontiguousarray((bq[None, cols] + q_embed[:, cols]) * rdk).astype(np.float32),
            "bk": np.ascontiguousarray(bk[None, cols] + k_embed[:, cols]).astype(np.float32),
            "bv": np.ascontiguousarray(bv[None, cols] + v_embed[:, cols]).astype(np.float32),
            "scales": scales,
            "wo": Wo.astype(_BF16),
            "lng": ln_g.astype(np.float32),
            "lnb": ln_b.astype(np.float32),
        })
    return in_maps


def kernel(**inputs):
    global LAST_RESULTS
    from concourse import bass_utils

    inputs = {k: np.asarray(v) for k, v in inputs.items()}
    if "nc" not in _CACHE:
        _CACHE["nc"] = _build()
    nc = _CACHE["nc"]

    in_maps = _prep_inputs(**inputs)
    res = bass_utils.run_bass_kernel_spmd(nc, in_maps, core_ids=list(range(NCORES)))
    LAST_RESULTS = res

    out = np.zeros((B, T, D), np.float32)
    for c in range(NCORES):
        b, r = divmod(c, 4)
        out[b, r * QS:(r + 1) * QS, :] = res.results[c]["out"]
    return out


# revision 29
# speedup vs baseline: 1.1928x; 1.0110x over previous
"""Trainium2 Bass kernel for nn_AdaptiveAttention (8-core SPMD).

Sharding: each core owns 2 heads (one 128-dim block of the QKV/head space)
for BOTH batches. Per core:
  - Q^T, K^T (bias folded, 1/sqrt(dk) folded into Wq) and V for its block,
  - full-T attention for its 4 (batch, head) units in the transposed-score
    layout S^T = [s, q] (lhsT=K^T slice, rhs=Q^T slice); softmax without
    max-subtraction (scores bounded ~|8|); per-batch adaptive scale applied
    via the Exp activation's scale operand; row-sums via ones-matmuls
    (column-packed); normalization folded into the AV PSUM->SBUF copy,
  - one 8-core AllToAll trades head-dim blocks for q-slices,
  - Wo matmul + residual + LayerNorm on its (batch, q-slice) of 512 rows.
Host side folds: sinusoidal PE into x, quantile MLP embeds into QKV biases,
1/sqrt(dk) into Wq/bq, bo into the residual. Core c outputs rows
[512*(c%4) : 512*(c%4+1)] of batch c//4.

All matmuls in bf16 (fp32 PSUM accumulate).
"""

import os
import numpy as np
import ml_dtypes

B, T, D = 2, 2048, 1024
H, DK = 16, 64
CPC = 128               # head-dim columns per core (2 heads)
QS = T // 4             # q-slice rows per output shard = 512
P = 128
NCORES = 8

_BF16 = ml_dtypes.bfloat16

_CACHE = {}
LAST_RESULTS = None


def _sinusoidal_pe(max_len, d_model):
    pos = np.arange(max_len)[:, None].astype(np.float32)
    div = np.exp(np.arange(0, d_model, 2).astype(np.float32) * (-np.log(10000.0) / d_model))
    pe = np.zeros((max_len, d_model), dtype=np.float32)
    pe[:, 0::2] = np.sin(pos * div)
    pe[:, 1::2] = np.cos(pos * div)
    return pe


def _build(taps=False):
    """Build + compile the SPMD Bass graph (one NEFF, runs on all 8 cores)."""
    import concourse.bass as bass
    import concourse.mybir as mybir
    import concourse.tile as tile
    from concourse import bacc

    f32 = mybir.dt.float32
    bf = mybir.dt.bfloat16
    Exp = mybir.ActivationFunctionType.Exp
    Sqrt = mybir.ActivationFunctionType.Sqrt
    sub = mybir.AluOpType.subtract
    mult = mybir.AluOpType.mult

    nc = bacc.Bacc("TRN2", target_bir_lowering=False, debug=False, num_devices=NCORES)

    xt_d = nc.dram_tensor("xt", [B, D, T], bf, kind="ExternalInput")
    xres_d = nc.dram_tensor("xres", [QS, D], f32, kind="ExternalInput")
    wq_d = nc.dram_tensor("wq", [D, CPC], bf, kind="ExternalInput")
    wk_d = nc.dram_tensor("wk", [B, D, CPC], bf, kind="ExternalInput")
    wv_d = nc.dram_tensor("wv", [D, CPC], bf, kind="ExternalInput")
    bq_d = nc.dram_tensor("bq", [B, CPC], f32, kind="ExternalInput")
    bk_d = nc.dram_tensor("bk", [B, CPC], f32, kind="ExternalInput")
    bv_d = nc.dram_tensor("bv", [B, CPC], f32, kind="ExternalInput")
    wo_d = nc.dram_tensor("wo", [D, D], bf, kind="ExternalInput")
    lng_d = nc.dram_tensor("lng", [D], f32, kind="ExternalInput")
    lnb_d = nc.dram_tensor("lnb", [D], f32, kind="ExternalInput")
    out_d = nc.dram_tensor("out", [QS, D], f32, kind="ExternalOutput")
    a2a_i = nc.dram_tensor("a2a_i", [NCORES, CPC, QS], bf, kind="Internal")
    a2a_o = nc.dram_tensor("a2a_o", [NCORES, CPC, QS], bf, kind="Internal")
    tap_d = {}
    if taps:
        tap_d["qt"] = nc.dram_tensor("tap_qt", [P, B, T], bf, kind="ExternalOutput")
        tap_d["kt"] = nc.dram_tensor("tap_kt", [P, B, T], bf, kind="ExternalOutput")
        tap_d["v"] = nc.dram_tensor("tap_v", [P, B, 16, 2, 65], bf, kind="ExternalOutput")
        tap_d["att"] = nc.dram_tensor("tap_att", [64, B, 2, T], bf, kind="ExternalOutput")
        tap_d["attf"] = nc.dram_tensor("tap_attf", [P, 8, QS], bf, kind="ExternalOutput")
        tap_d["rs"] = nc.dram_tensor("tap_rs", [4, 4, 512], f32, kind="ExternalOutput")

    def bcast_ap(src, nparts):
        """Partition-broadcast DMA source AP from a 1-partition AP."""
        return bass.AP(
            tensor=src.tensor,
            offset=src.offset,
            ap=[[0, nparts]] + [list(d) for d in src.ap[1:]],
        )

    with tile.TileContext(nc) as tc:
        with tc.tile_pool(name="const", bufs=1) as const:
            # ---- load inputs (weights first so matmuls start early;
            # spread across the sync/scalar/gpsimd DMA queues) ----
            qeng = [nc.sync, nc.scalar, nc.gpsimd]
            xt_sb = const.tile([P, B, 8, T], bf)
            xt_ap = xt_d.ap()
            wq_sb = const.tile([P, 8, CPC], bf)
            wk_sb = const.tile([P, B, 8, CPC], bf)
            wv_sb = const.tile([P, 8, CPC], bf)
            for k in range(8):
                qeng[k % 3].dma_start(out=wq_sb[:, k, :], in_=wq_d.ap()[k * P:(k + 1) * P, :])
            for b in range(B):
                for k in range(8):
                    qeng[k % 3].dma_start(
                        out=wk_sb[:, b, k, :], in_=wk_d.ap()[b, k * P:(k + 1) * P, :]
                    )
            for k in range(8):
                qeng[k % 3].dma_start(out=xt_sb[:, 0, k, :], in_=xt_ap[0, k * P:(k + 1) * P, :])
            for k in range(8):
                qeng[k % 3].dma_start(out=wv_sb[:, k, :], in_=wv_d.ap()[k * P:(k + 1) * P, :])
            for k in range(8):
                qeng[k % 3].dma_start(out=xt_sb[:, 1, k, :], in_=xt_ap[1, k * P:(k + 1) * P, :])
            w_sbs = {"q": wq_sb, "v": wv_sb}

            wo_sb = const.tile([P, 8, D], bf)
            wo_ap = wo_d.ap()
            for k in range(8):
                qeng[k % 3].dma_start(out=wo_sb[:, k, :], in_=wo_ap[k * P:(k + 1) * P, :])

            bq_sb = const.tile([P, B], f32)
            bk_sb = const.tile([P, B], f32)
            nc.sync.dma_start(out=bq_sb[:], in_=bq_d.ap().rearrange("b p -> p b"))
            nc.sync.dma_start(out=bk_sb[:], in_=bk_d.ap().rearrange("b p -> p b"))
            bv_bc = const.tile([P, B, CPC], f32)
            for b in range(B):
                nc.sync.dma_start(out=bv_bc[:, b, :], in_=bcast_ap(bv_d.ap()[b:b + 1, :], P))
            lng_b = const.tile([P, D], f32)
            lnb_b = const.tile([P, D], f32)
            nc.sync.dma_start(out=lng_b[:], in_=bcast_ap(lng_d.ap()[None, :], P))
            nc.sync.dma_start(out=lnb_b[:], in_=bcast_ap(lnb_d.ap()[None, :], P))
            xres_sb = const.tile([P, 4, D], f32)
            xres_ap = xres_d.ap()
            for m in range(4):
                nc.sync.dma_start(out=xres_sb[:, m, :], in_=xres_ap[m * P:(m + 1) * P, :])

            eps_sb = const.tile([P, 1], f32)
            nc.vector.memset(eps_sb[:], 1e-5)

            QT_sb = const.tile([P, B, T], bf)
            KT_sb = const.tile([P, B, T], bf)
            # V augmented with a ones-column per head: free layout [b, schunk, hh, 65]
            # col 64 == 1.0 so the AV matmul's output row 64 is the exp row-sum.
            V_sb = const.tile([P, B, 16, 2, 65], bf)
            nc.vector.memset(V_sb[:], 1.0)
            attT_sb = const.tile([64, B, 2, T], bf)
            attTf_sb = const.tile([P, 8, QS], bf)

            # ---- phase 1: projections ----
            with tc.tile_pool(name="qk_ps", bufs=4, space="PSUM") as qk_ps, \
                 tc.tile_pool(name="v_ps", bufs=4, space="PSUM") as v_ps:
                for wname, b_sb, dst in (("q", bq_sb, QT_sb), ("k", bk_sb, KT_sb)):
                    for b in range(B):
                        for n in range(4):
                            ps = qk_ps.tile([P, 512], f32, tag="qkps")
                            for k in range(8):
                                lhsT = (wq_sb[:, k, :] if wname == "q"
                                        else wk_sb[:, b, k, :])
                                nc.tensor.matmul(
                                    ps[:],
                                    lhsT=lhsT,
                                    rhs=xt_sb[:, b, k, n * 512:(n + 1) * 512],
                                    start=(k == 0), stop=(k == 7),
                                )
                            nc.vector.tensor_scalar_add(
                                out=dst[:, b, n * 512:(n + 1) * 512],
                                in0=ps[:], scalar1=b_sb[:, b:b + 1],
                            )
                for b in range(B):
                    for mt in range(16):
                        ps = v_ps.tile([P, CPC], f32, tag="vps")
                        for k in range(8):
                            nc.tensor.matmul(
                                ps[:],
                                lhsT=xt_sb[:, b, k, mt * P:(mt + 1) * P],
                                rhs=w_sbs["v"][:, k, :],
                                start=(k == 0), stop=(k == 7),
                            )
                        for hh in range(2):
                            hsl = slice(64 * hh, 64 * (hh + 1))
                            nc.vector.tensor_add(
                                out=V_sb[:, b, mt, hh, 0:64],
                                in0=ps[:, hsl], in1=bv_bc[:, b, hsl],
                            )

            # ---- phase 2: attention (S^T layout) ----
            with tc.tile_pool(name="sp_ps", bufs=2, space="PSUM") as sp_ps, \
                 tc.tile_pool(name="av_ps", bufs=4, space="PSUM") as av_ps, \
                 tc.tile_pool(name="et_pool", bufs=4) as et_pool, \
                 tc.tile_pool(name="avs_pool", bufs=6) as avs_pool, \
                 tc.tile_pool(name="rc_pool", bufs=4) as rc_pool, \
                 tc.tile_pool(name="rcd_pool", bufs=2, space="DRAM") as rcd_pool, \
                 tc.tile_pool(name="rb_pool", bufs=4) as rb_pool:
                for qb in range(4):
                    qsl = slice(qb * 512, (qb + 1) * 512)
                    avs = [av_ps.tile([65, 512], f32, tag="av", name=f"av{qb}_{u}")
                           for u in range(4)]
                    for sc in range(16):
                        for b in range(B):
                            # one 2-bank PSUM tile holds both heads' scores;
                            # the two matmuls use distinct row groups (h0 rows
                            # 0-63, h1 rows 64-127) and run concurrently.
                            sp = sp_ps.tile([P, 2, 512], f32, tag="sp")
                            nc.tensor.matmul(
                                sp[:, 0, :],
                                lhsT=KT_sb[0:64, b, sc * P:(sc + 1) * P],
                                rhs=QT_sb[0:64, b, qsl],
                                start=True, stop=True,
                            )
                            nc.tensor.matmul(
                                sp[:, 1, :],
                                lhsT=KT_sb[64:128, b, sc * P:(sc + 1) * P],
                                rhs=QT_sb[64:128, b, qsl],
                                start=True, stop=True,
                            )
                            et = et_pool.tile([P, 2, 512], bf, tag="et")
                            nc.scalar.activation(out=et[:], in_=sp[:], func=Exp)
                            for hh in range(2):
                                nc.tensor.matmul(
                                    avs[2 * b + hh][:],
                                    lhsT=V_sb[:, b, sc, hh, :],
                                    rhs=et[:, hh, :],
                                    start=(sc == 0), stop=(sc == 15),
                                )
                    # decouple: drain AV PSUM to SBUF so the next q-block's
                    # matmuls can reuse the banks; normalization happens from
                    # SBUF off the PE critical path.
                    av_sb = [avs_pool.tile([65, 512], f32, tag="avsb", name=f"avsb{qb}_{u}")
                             for u in range(4)]
                    for u in range(4):
                        nc.vector.tensor_copy(out=av_sb[u][:], in_=avs[u][:])
                    # transpose-trick reciprocal: bounce the 4 [1,512] row-sum
                    # rows through DRAM into a [128,16] tile so the reciprocal
                    # uses all 128 DVE lanes, then scatter 1/rs back to DRAM
                    # for the partition-broadcast loads.
                    rsd = rcd_pool.tile([4, 512], f32, tag="rsd")
                    for u in range(4):
                        nc.sync.dma_start(out=rsd[u:u + 1, :], in_=av_sb[u][64:65, :])
                    rst = rc_pool.tile([P, 16], f32, tag="rst")
                    nc.sync.dma_start(
                        out=rst[:], in_=rsd[:].rearrange("u (a p) -> p (u a)", p=P)
                    )
                    rct = rc_pool.tile([P, 16], f32, tag="rct")
                    nc.vector.reciprocal(out=rct[:], in_=rst[:])
                    rcd = rcd_pool.tile([4, 512], f32, tag="rcd")
                    nc.sync.dma_start(
                        out=rcd[:].rearrange("u (a p) -> p (u a)", p=P), in_=rct[:]
                    )
                    if taps:
                        nc.sync.dma_start(out=tap_d["rs"].ap()[qb], in_=rcd[:])
                    for b in range(B):
                        for hh in range(2):
                            u = 2 * b + hh
                            rb = rb_pool.tile([64, 512], f32, tag="rb")
                            nc.sync.dma_start(
                                out=rb[:], in_=bcast_ap(rcd[u:u + 1, :], 64),
                            )
                            nc.vector.tensor_mul(
                                out=attT_sb[:, b, hh, qsl], in0=av_sb[u][0:64, :], in1=rb[:],
                            )
                    for j2 in range(2):  # the two a2a slots covering this qb
                        j = j2 * 4 + qb
                        for hh in range(2):
                            nc.sync.dma_start(
                                out=a2a_i.ap()[j, 64 * hh:64 * (hh + 1), :],
                                in_=attT_sb[:, j2, hh, qsl],
                            )

            if taps:
                for b in range(B):
                    nc.sync.dma_start(out=tap_d["qt"].ap()[:, b, :], in_=QT_sb[:, b, :])
                    nc.sync.dma_start(out=tap_d["kt"].ap()[:, b, :], in_=KT_sb[:, b, :])
                    for hh in range(2):
                        nc.sync.dma_start(
                            out=tap_d["att"].ap()[:, b, hh, :], in_=attT_sb[:, b, hh, :]
                        )
                    for mt in range(16):
                        nc.sync.dma_start(
                            out=tap_d["v"].ap()[:, b, mt, :, :], in_=V_sb[:, b, mt, :, :]
                        )

            # ---- phase 3: all-to-all + Wo + residual + LN ----
            nc.gpsimd.collective_compute(
                "AllToAll",
                mybir.AluOpType.bypass,
                replica_groups=[list(range(NCORES))],
                ins=[a2a_i.ap()],
                outs=[a2a_o.ap()],
            )
            for j in range(8):
                qeng[j % 3].dma_start(out=attTf_sb[:, j, :], in_=a2a_o.ap()[j, :, :])
            if taps:
                for j in range(8):
                    nc.sync.dma_start(out=tap_d["attf"].ap()[:, j, :], in_=attTf_sb[:, j, :])

            with tc.tile_pool(name="wo_ps", bufs=4, space="PSUM") as wo_ps, \
                 tc.tile_pool(name="y_pool", bufs=3) as y_pool, \
                 tc.tile_pool(name="stat", bufs=4) as stat:
                for m in range(4):
                    y = y_pool.tile([P, D], f32, tag="y")
                    for n in range(2):
                        nsl = slice(n * 512, (n + 1) * 512)
                        ps = wo_ps.tile([P, 512], f32, tag="wops")
                        for k in range(8):
                            nc.tensor.matmul(
                                ps[:],
                                lhsT=attTf_sb[:, k, m * P:(m + 1) * P],
                                rhs=wo_sb[:, k, nsl],
                                start=(k == 0), stop=(k == 7),
                            )
                        nc.vector.tensor_add(out=y[:, nsl], in0=ps[:], in1=xres_sb[:, m, nsl])
                    st = stat.tile([P, 2, 6], f32, tag="st")
                    nc.vector.bn_stats(out=st[:, 0, :], in_=y[:, 0:512])
                    nc.vector.bn_stats(out=st[:, 1, :], in_=y[:, 512:1024])
                    mv = stat.tile([P, 2], f32, tag="mv")
                    nc.vector.bn_aggr(out=mv[:], in_=st[:])
                    std = stat.tile([P, 1], f32, tag="std")
                    nc.scalar.activation(out=std[:], in_=mv[:, 1:2], func=Sqrt, bias=eps_sb[:])
                    rstd = stat.tile([P, 1], f32, tag="rstd")
                    nc.vector.reciprocal(out=rstd[:], in_=std[:])
                    nc.vector.tensor_scalar(
                        out=y[:], in0=y[:], scalar1=mv[:, 0:1], scalar2=rstd[:],
                        op0=sub, op1=mult,
                    )
                    nc.vector.tensor_mul(out=y[:], in0=y[:], in1=lng_b[:])
                    nc.vector.tensor_add(out=y[:], in0=y[:], in1=lnb_b[:])
                    qeng[m % 3].dma_start(out=out_d.ap()[m * P:(m + 1) * P, :], in_=y[:])

    nc.compile()
    return nc


def _prep_inputs(x, quantile, quantile_importance,
                 Wq, bq, Wk, bk, Wv, bv, Wo, bo,
                 qpq_w1, qpq_b1, qpq_w2, qpq_b2,
                 qpk_w1, qpk_b1, qpk_w2, qpk_b2,
                 qpv_w1, qpv_b1, qpv_w2, qpv_b2,
                 ln_g, ln_b):
    pe = _sinusoidal_pe(T, D)
    xp = x.astype(np.float32) + pe[None]

    q = quantile.astype(np.float32)

    def mlp(w1, b1, w2, b2):
        return np.maximum(q @ w1 + b1, 0.0) @ w2 + b2

    q_embed = mlp(qpq_w1, qpq_b1, qpq_w2, qpq_b2)
    k_embed = mlp(qpk_w1, qpk_b1, qpk_w2, qpk_b2)
    v_embed = mlp(qpv_w1, qpv_b1, qpv_w2, qpv_b2)

    buf = quantile_importance.astype(np.float32)
    idx = np.clip((q[:, 0] * 100).astype(np.int32), 0, 99)
    mx = buf.max()
    imp = buf[idx]
    imp = np.where(mx > 0, imp / mx, imp)
    scales = (1.0 + imp).astype(np.float32)          # [B], folded into Wk/bk
    rdk = np.float32(1.0 / np.sqrt(DK))              # folded into Wq/bq

    xt_all = np.ascontiguousarray(np.transpose(xp, (0, 2, 1))).astype(_BF16)  # [B, D, T]
    in_maps = []
    for c in range(NCORES):
        b, r = divmod(c, 4)
        cols = slice(c * CPC, (c + 1) * CPC)
        qsl = slice(r * QS, (r + 1) * QS)
        in_maps.append({
            "xt": xt_all,
            "xres": np.ascontiguousarray(xp[b, qsl] + bo[None, :]).astype(np.float32),
            "wq": np.ascontiguousarray(Wq[:, cols] * rdk).astype(_BF16),
            "wk": np.ascontiguousarray(Wk[None, :, cols] * scales[:, None, None]).astype(_BF16),
            "wv": np.ascontiguousarray(Wv[:, cols]).astype(_BF16),
            "bq": np.ascontiguousarray((bq[None, cols] + q_embed[:, cols]) * rdk).astype(np.float32),
            "bk": np.ascontiguousarray((bk[None, cols] + k_embed[:, cols]) * scales[:, None]).astype(np.float32),
            "bv": np.ascontiguousarray(bv[None, cols] + v_embed[:, cols]).astype(np.float32),
            "wo": Wo.astype(_BF16),
            "lng": ln_g.astype(np.float32),
            "lnb": ln_b.astype(np.float32),
        })
    return in_maps


def kernel(**inputs):
    global LAST_RESULTS
    from concourse import bass_utils

    inputs = {k: np.asarray(v) for k, v in inputs.items()}
    if "nc" not in _CACHE:
        _CACHE["nc"] = _build()
    nc = _CACHE["nc"]

    in_maps = _prep_inputs(**inputs)
    res = bass_utils.run_bass_kernel_spmd(nc, in_maps, core_ids=list(range(NCORES)))
    LAST_RESULTS = res

    out = np.zeros((B, T, D), np.float32)
    for c in range(NCORES):
        b, r = divmod(c, 4)
        out[b, r * QS:(r + 1) * QS, :] = res.results[c]["out"]
    return out
